# revision 26
# baseline (speedup 1.0000x reference)
"""Multi-head cross-attention on 8 Trainium2 NeuronCores.

Problem shapes (hardcoded): B=4, Ld=1024, Le=2048, d_model=1024, 8 heads x 128.
Sharding: core c handles batch b=c//2 and head-group g=c%2 (4 heads each).
Each core computes q/k/v projections for its heads, attention, and a partial
output projection over its heads' value dims; the host sums the two partial
outputs per batch and adds b_o.

Projections run as split-fp8 DoubleRow matmuls: every operand is host-split
into hi+lo fp8e4 parts (hi = fp8(s*x), lo = fp8(s*x - hi), one power-of-2
scale s per tensor chosen for e4m3's normal range).  A DoubleRow matmul
contracts two 128-partition chunks per pass at 0.5 cycles/row, and the three
products hi*hi, hi*lo, lo*hi (lo*lo is ~2^-8 relative, dropped) cover a
d-chunk pair in 3 matmuls = 0.75x the bf16 cost at bf16-class accuracy.
PSUM drains rescale by the product of the operand scales (fused into the
bias-add on the DVE).  The output projection does the same with vals split
on-chip by the normalize chain.

All inputs are host-repacked into contiguous [128, ...] layouts so each
tensor is one DMA (descriptor-generation slots are an exclusive ~625ns
resource).  DMA order feeds the V path first (wk, enc, wv) with hi parts
before lo parts, giving the PE continuous early work while wq/x stream in.

Schedule: one software-pipelined stream; projection bank-groups (K/Q/V and
later the output projection) are queued a few per attention chunk-pair so
the PE never waits for the Act engine's exp stream.

Softmax denominators use tall-skinny matmuls: pT is the *stationary* operand
and a ones column moves, so each [128,1] per-query partial sum costs ~1 PE
row instead of the 512 a [1,512] ones-stationary layout costs.

Exps are issued per chunk-pair over a [128,1024] two-bank PSUM span (halving
the Act engine's fixed access overhead), double-buffered across two spans so
scores never wait on the previous exp.

The per-stage normalize chain (reciprocal -> PE-transpose to a [1,512] row
in a rotating projection bank -> gpsimd partition-broadcast -> scaled
multiply + fp8 hi/lo split of vals) is split: the PV accumulator is drained
to SBUF raw (one DVE copy) so the next stage's PV can start immediately, and
the rest is emitted after the next stage's first pair, off the PE's
critical path.

PSUM banks: 0-3 two score pair-spans, 4 PV accumulator, 5 denominator
columns, 6-7 projection / output-projection accumulators (rotating).
"""

import math
import sys

import numpy as np

for _p in ("/opt/trn_rl_repo", "/root/.axon_site/_ro/trn_rl_repo"):
    if _p not in sys.path:
        sys.path.append(_p)

B = 4
LQ = 1024
LK = 2048
D = 1024
H = 8
DH = 128
P = 128
HPC = 4          # heads per core
OQ = HPC * DH    # 512 projected dims per core
NQ = 512         # matmul moving free dim
KC = D // P      # 8 contraction chunks for projections
LKC = LK // P    # 16 key chunks
N_CORES = 8

# fp8e4 (e4m3) per-tensor scales: picked so values sit in the normal range.
SA = 8.0       # x, enc ~ N(0,1)
SW = 128.0     # wk, wv (sigma 1/32)
SQ = 1024.0    # wq with 1/sqrt(dh) folded (sigma 1/256)
SO = 128.0     # wo (sigma 1/32)
SVAL = 32.0    # vals (sigma ~0.03), applied on-chip
KINV = 1.0 / (SA * SW)
VINV = 1.0 / (SA * SW)
QINV = 1.0 / (SA * SQ)
OINV = 1.0 / (SVAL * SO)

_BUILT = {}


def _repack(a):
    """[KC*128, X] -> [128, KC*X] with d-chunk-major columns."""
    kc = a.shape[0] // P
    return np.ascontiguousarray(
        a.reshape(kc, P, a.shape[1]).transpose(1, 0, 2).reshape(P, -1))


def _split8(a, s):
    import ml_dtypes
    f8 = ml_dtypes.float8_e4m3
    sa = (a * s).astype(np.float32)
    hi = sa.astype(f8)
    lo = (sa - hi.astype(np.float32)).astype(f8)
    return hi, lo


def _build(masked):
    import concourse.bass as bass  # noqa: F401
    import concourse.tile as tile
    import concourse.mybir as mybir
    from concourse import bacc

    f32 = mybir.dt.float32
    bf16 = mybir.dt.bfloat16
    f32r = mybir.dt.float32r
    fp8 = mybir.dt.float8e4
    DR = mybir.MatmulPerfMode.DoubleRow
    Exp = mybir.ActivationFunctionType.Exp
    MUL = mybir.AluOpType.mult
    ADD = mybir.AluOpType.add
    SUB = mybir.AluOpType.subtract

    nc = bacc.Bacc("TRN2", target_bir_lowering=False, debug=False,
                   num_devices=N_CORES)

    def din(name, shape, dt=fp8):
        return nc.dram_tensor(name, shape, dt, kind="ExternalInput").ap()

    x_d = [[din(f"x{q}{p}", [P, KC, NQ]) for p in range(2)] for q in range(2)]
    e_d = [[din(f"enc{q}{p}", [P, KC, NQ]) for p in range(2)]
           for q in range(4)]
    wk_d = [din(f"wk{p}", [P, KC, OQ]) for p in range(2)]
    wv_d = [din(f"wv{p}", [P, KC, OQ]) for p in range(2)]
    wq_d = [din(f"wq{p}", [P, KC, OQ]) for p in range(2)]
    wo_d = [din(f"wo{p}", [P, HPC, D]) for p in range(2)]
    # bk | bq | bvb | ident packed as one f32 DMA.
    smf_d = din("smf", [P, 648], f32)
    ones_d = din("ones", [P, 1], bf16)
    if masked:
        maskT = din("maskT", [LK, LQ], f32)
    out_d = nc.dram_tensor("out", [LQ, D], bf16, kind="ExternalOutput").ap()

    with tile.TileContext(nc) as tc:
        with tc.tile_pool(name="persist", bufs=1) as persist:
            qT = [[persist.tile([P, NQ], f32r, name=f"qT{h}_{q}")
                   for q in range(2)] for h in range(HPC)]
            kT = [[persist.tile([P, NQ], f32r, name=f"kT{h}_{lk}")
                   for lk in range(4)] for h in range(HPC)]
            vch = [persist.tile([P, OQ], bf16, name=f"v{j}") for j in range(LKC)]
            smf = persist.tile([P, 648], f32, name="smf")
            bk_sb = smf[:, 0:HPC]
            bq_sb = smf[:, HPC:2 * HPC]
            bv_sb = smf[:, 2 * HPC:2 * HPC + OQ]
            ident = smf[:, 2 * HPC + OQ:2 * HPC + OQ + P]
            ones_col = persist.tile([P, 1], bf16, name="ones")
            warm = persist.tile([1, HPC], f32, name="warm")
            wkb = [persist.tile([P, KC, OQ], fp8, name=f"wkb{p}")
                   for p in range(2)]
            wvb = [persist.tile([P, KC, OQ], fp8, name=f"wvb{p}")
                   for p in range(2)]
            wqb = [persist.tile([P, KC, OQ], fp8, name=f"wqb{p}")
                   for p in range(2)]
            wob = [persist.tile([P, HPC, D], fp8, name=f"wob{p}")
                   for p in range(2)]
            eb = [[persist.tile([P, KC, NQ], fp8, name=f"eb{q}_{p}")
                   for p in range(2)] for q in range(4)]
            xb = [[persist.tile([P, KC, NQ], fp8, name=f"xb{q}_{p}")
                   for p in range(2)] for q in range(2)]
            vals = [[persist.tile([P, HPC, NQ], fp8, name=f"vals{q}_{p}")
                     for p in range(2)] for q in range(2)]

            with (
                tc.tile_pool(name="acc", bufs=1, space="PSUM") as acc,
                tc.tile_pool(name="pTp", bufs=3) as pTp,
                tc.tile_pool(name="smallp", bufs=2) as smallp,
                tc.tile_pool(name="maskp", bufs=16 if masked else 1) as maskp,
                tc.tile_pool(name="osb", bufs=4) as osb,
            ):
                # PSUM: two score pair-spans (banks 0-3), single PV
                # accumulator (bank 4), denominator columns (bank 5),
                # projection/out-proj accumulators (banks 6-7).
                sp = [acc.tile([P, 2 * NQ], f32, name=f"sp{t}")
                      for t in range(2)]
                pvb = acc.tile([P, NQ], f32, name="pvb")
                dbk = acc.tile([P, NQ], f32, name="dbk")
                pj = [acc.tile([P, NQ], f32, name=f"pj{t}") for t in range(2)]

                # Warm-up fill: [1,512]-out matmuls against a memset tile (no
                # DMA dependency) keep the PE busy through the DMA lead-in
                # and hold the p-state ramp continuous; the Act Exp table is
                # preloaded the same way.
                wt = persist.tile([P, NQ], bf16, name="wt")
                nc.vector.memset(wt[:], 1.0)

                def warmfill(n):
                    for _ in range(n):
                        nc.tensor.matmul(pj[1][:1, :], wt[:, :1], wt[:],
                                         start=True, stop=True)

                warmfill(2)
                nc.scalar.activation(warm[:], wt[:1, :HPC], Exp)
                # ---- DMA issue order == service order: V path first,
                # hi parts before lo parts.
                nc.sync.dma_start(wkb[0][:], wk_d[0][:])
                nc.sync.dma_start(eb[0][0][:], e_d[0][0][:])
                nc.sync.dma_start(wkb[1][:], wk_d[1][:])
                nc.sync.dma_start(eb[0][1][:], e_d[0][1][:])
                nc.sync.dma_start(smf[:], smf_d[:])
                nc.sync.dma_start(ones_col[:], ones_d[:])
                for p in range(2):
                    nc.sync.dma_start(wvb[p][:], wv_d[p][:])
                for p in range(2):
                    nc.sync.dma_start(eb[1][p][:], e_d[1][p][:])
                for p in range(2):
                    nc.sync.dma_start(wqb[p][:], wq_d[p][:])
                for p in range(2):
                    nc.sync.dma_start(xb[0][p][:], x_d[0][p][:])
                for p in range(2):
                    nc.sync.dma_start(eb[2][p][:], e_d[2][p][:])
                for p in range(2):
                    nc.sync.dma_start(eb[3][p][:], e_d[3][p][:])
                for p in range(2):
                    nc.sync.dma_start(xb[1][p][:], x_d[1][p][:])
                for p in range(2):
                    nc.sync.dma_start(wob[p][:], wo_d[p][:])

                # ---- projection bank-group emitters (banks 6-7 rotating).
                # Each contraction d-chunk pair takes 3 DoubleRow matmuls:
                # hi*hi, lo(w)*hi, hi*lo (lo*lo dropped).  The hi*hi pass for
                # all pairs is emitted first so it can start as soon as the
                # hi DMAs land.
                nbg = [0]

                def next_pj():
                    bank = pj[nbg[0] % 2]
                    nbg[0] += 1
                    return bank

                def dr_group(bank, wts, mov, wslice, mslice, between=None):
                    """wts/mov: [hi, lo] tile lists; slices by d-pair dp."""
                    np_ = KC // 2
                    for wp, mp, first, last in ((0, 0, True, False),
                                                (1, 0, False, False),
                                                (0, 1, False, True)):
                        for dp in range(np_):
                            nc.tensor.matmul(
                                bank[:],
                                wts[wp][wslice(dp)],
                                mov[mp][mslice(dp)],
                                start=(first and dp == 0),
                                stop=(last and dp == np_ - 1),
                                perf_mode=DR)
                        if between is not None and not last:
                            between()

                def kproj_group(h, lk, between=None):
                    bank = next_pj()
                    dr_group(
                        bank, wkb, eb[lk],
                        lambda dp: np.s_[:, 2 * dp:2 * dp + 2,
                                         h * DH:(h + 1) * DH],
                        lambda dp: np.s_[:, 2 * dp:2 * dp + 2, :],
                        between=between)
                    nc.vector.tensor_scalar(
                        kT[h][lk][:], bank[:], KINV, bk_sb[:, h:h + 1],
                        MUL, ADD)

                def qproj_group(h, q2):
                    bank = next_pj()
                    dr_group(
                        bank, wqb, xb[q2],
                        lambda dp: np.s_[:, 2 * dp:2 * dp + 2,
                                         h * DH:(h + 1) * DH],
                        lambda dp: np.s_[:, 2 * dp:2 * dp + 2, :])
                    nc.vector.tensor_scalar(
                        qT[h][q2][:], bank[:], QINV, bq_sb[:, h:h + 1],
                        MUL, ADD)

                def vproj_group(j):
                    bank = next_pj()
                    dr_group(
                        bank, eb[j // 4], wvb,
                        lambda dp: np.s_[:, 2 * dp:2 * dp + 2,
                                         (j % 4) * P:(j % 4 + 1) * P],
                        lambda dp: np.s_[:, 2 * dp:2 * dp + 2, :])
                    nc.vector.scalar_tensor_tensor(
                        vch[j][:], bank[:], VINV, bv_sb[:], MUL, ADD)

                def oproj_group(lqc, o2, n):
                    bank = next_pj()
                    q2g = lqc // 4
                    for wp, mp, first, last in ((0, 0, True, False),
                                                (1, 0, False, False),
                                                (0, 1, False, True)):
                        for hp in range(HPC // 2):
                            nc.tensor.matmul(
                                bank[:],
                                vals[q2g][wp][:, 2 * hp:2 * hp + 2,
                                              (lqc % 4) * P:
                                              (lqc % 4 + 1) * P],
                                wob[mp][:, 2 * hp:2 * hp + 2,
                                        o2 * NQ:(o2 + 1) * NQ],
                                start=(first and hp == 0),
                                stop=(last and hp == HPC // 2 - 1),
                                perf_mode=DR)
                    ot = osb.tile([P, NQ], bf16, name="ot")
                    if n % 2 == 0:
                        nc.vector.tensor_scalar_mul(ot[:], bank[:], OINV)
                    else:
                        nc.scalar.mul(ot[:], bank[:], OINV)
                    nc.sync.dma_start(
                        out_d[lqc * P:(lqc + 1) * P,
                              o2 * NQ:(o2 + 1) * NQ], ot[:])

                # ---- attention emitters
                mask_tiles = [None] * LKC

                def attn_pair(q2, h, jp):
                    """Chunks j=2jp,2jp+1: scores into pair-span jp%2, one
                    exp over both, then PV + denominator matmuls."""
                    span = sp[jp % 2]
                    for t in range(2):
                        j = 2 * jp + t
                        nc.tensor.matmul(
                            span[:, t * NQ:(t + 1) * NQ],
                            kT[h][j // 4][:, (j % 4) * P:(j % 4 + 1) * P],
                            qT[h][q2][:],
                            start=True, stop=True)
                        if masked:
                            nc.vector.tensor_add(
                                span[:, t * NQ:(t + 1) * NQ],
                                span[:, t * NQ:(t + 1) * NQ],
                                mask_tiles[j][:])
                    pT2 = pTp.tile([P, 2 * NQ], bf16, name="pT2")
                    nc.scalar.activation(pT2[:], span[:], Exp)
                    for t in range(2):
                        j = 2 * jp + t
                        nc.tensor.matmul(
                            pvb[:],
                            vch[j][:, h * DH:(h + 1) * DH],
                            pT2[:, t * NQ:(t + 1) * NQ],
                            start=(j == 0), stop=(j == LKC - 1))
                        for s in range(4):
                            nc.tensor.matmul(
                                dbk[:, s:s + 1],
                                pT2[:, t * NQ + s * P:t * NQ + (s + 1) * P],
                                ones_col[:],
                                start=(j == 0 and s == 0),
                                stop=(j == LKC - 1 and s == 3),
                                skip_group_check=True)

                def attn_norm_start(q2, h):
                    """Free the PV/denominator banks: raw-copy the PV
                    accumulator and take the reciprocal of d."""
                    pvraw = smallp.tile([P, NQ], f32, name="pvraw")
                    nc.vector.tensor_copy(pvraw[:], pvb[:])
                    rsb = smallp.tile([P, 4], f32, name="rsb")
                    nc.vector.reciprocal(rsb[:], dbk[:, 0:4])
                    return pvraw, rsb

                def attn_norm_finish(q2, h, pvraw, rsb):
                    """1/d -> [1,512] row via PE transposes (into a rotating
                    projection bank) -> partition broadcast -> scaled
                    normalize, split into hi+lo fp8 for the out-proj."""
                    tb = next_pj()
                    for s in range(4):
                        nc.tensor.transpose(
                            tb[0:1, s * P:(s + 1) * P],
                            rsb[:, s:s + 1], ident[:])
                    rrow = smallp.tile([1, NQ], f32, name="rrow")
                    nc.vector.tensor_copy(rrow[:], tb[0:1, :])
                    bcast = smallp.tile([P, NQ], f32, name="bcast")
                    nc.gpsimd.partition_broadcast(bcast[:], rrow[:])
                    t2 = smallp.tile([P, NQ], f32, name="t2")
                    nc.vector.scalar_tensor_tensor(
                        t2[:], pvraw[:], SVAL, bcast[:], MUL, MUL)
                    vh = vals[q2][0][:, h:h + 1, :]
                    nc.vector.tensor_copy(vh, t2[:])
                    nc.vector.scalar_tensor_tensor(
                        vals[q2][1][:, h:h + 1, :], t2[:], 1.0, vh, MUL, SUB)

                # ---- the pipelined schedule.
                # Pre-stage: first K group, the V stream, then Q(h0,0), with
                # warm-up fill sized to the DMA lead-in.
                warmfill(14)
                kproj_group(0, 0, between=lambda: warmfill(3))
                warmfill(6)
                for j in range(8):
                    vproj_group(j)
                qproj_group(0, 0)

                K, Q, V = kproj_group, qproj_group, vproj_group

                def O(n):
                    return lambda: oproj_group(n // 2, n % 2, n)

                def L(f, *a):
                    return lambda: f(*a)

                # Per-stage, per-pair-slot work placement.  Projections lead
                # the attention that consumes them; out-proj groups go at
                # jp>=5 of their earliest stage or jp=0 of later stages so
                # the producing normalize chain has always landed; each
                # attention-only stage start gets one filler group to cover
                # the exp pipeline-fill bubble.
                stage_work = {
                    0: {0: [L(K, 0, 1), L(V, 8)], 1: [L(V, 9), L(K, 0, 2)],
                        2: [L(V, 10), L(V, 11)], 3: [L(K, 0, 3), L(V, 12)],
                        4: [L(V, 13), L(K, 1, 0)], 5: [L(V, 14), L(V, 15)],
                        6: [L(K, 1, 1), L(K, 1, 2)],
                        7: [L(K, 1, 3), L(Q, 1, 0)]},
                    1: {0: [L(Q, 0, 1)], 1: [L(Q, 1, 1)], 2: [L(K, 2, 0)],
                        3: [L(K, 2, 1)], 4: [L(K, 2, 2)], 5: [L(K, 2, 3)],
                        6: [L(Q, 2, 0)]},
                    2: {0: [L(K, 3, 0)], 1: [L(K, 3, 1)], 2: [L(K, 3, 2)],
                        3: [L(K, 3, 3)], 4: [L(Q, 3, 0)]},
                    3: {0: [L(Q, 2, 1)]},
                    4: {0: [L(Q, 3, 1)], 5: [O(0)], 6: [O(1)]},
                    5: {0: [O(2)], 5: [O(3)]},
                    6: {0: [O(4)], 5: [O(5)]},
                    7: {0: [O(6)], 5: [O(7)]},
                }

                pending = None
                for idx in range(8):
                    q2, h = idx // 4, idx % 4
                    if masked and h == 0:
                        for j in range(LKC):
                            mt = maskp.tile([P, NQ], f32, name=f"m{j}")
                            nc.sync.dma_start(
                                mt[:],
                                maskT[j * P:(j + 1) * P,
                                      q2 * NQ:(q2 + 1) * NQ])
                            mask_tiles[j] = mt
                    work = stage_work.get(idx, {})
                    for jp in range(LKC // 2):
                        for w in work.get(jp, []):
                            w()
                        attn_pair(q2, h, jp)
                        if jp == 0 and pending is not None:
                            # finish the previous stage's normalize off the
                            # critical path
                            attn_norm_finish(*pending)
                            pending = None
                    pvraw, rsb = attn_norm_start(q2, h)
                    pending = (q2, h, pvraw, rsb)

                attn_norm_finish(*pending)
                # Tail: second-half output projection.
                for n in range(8, 16):
                    oproj_group(n // 2, n % 2, n)

    nc.compile()
    return nc


def _get_built(masked):
    if masked not in _BUILT:
        _BUILT[masked] = _build(masked)
    return _BUILT[masked]


def _shard_inputs(inputs, masked):
    import ml_dtypes

    bf16 = ml_dtypes.bfloat16

    x = np.asarray(inputs["mhca_input"], np.float32)
    enc = np.asarray(inputs["encoder_output"], np.float32)
    mask = np.asarray(inputs["cross_mask"], np.float32)
    W_kv = np.asarray(inputs["W_kv"], np.float32)
    b_kv = np.asarray(inputs["b_kv"], np.float32)
    W_q = np.asarray(inputs["W_q"], np.float32)
    b_q = np.asarray(inputs["b_q"], np.float32)
    W_o = np.asarray(inputs["W_o"], np.float32)

    scale = 1.0 / math.sqrt(DH)
    in_maps = []
    for c in range(N_CORES):
        b = c // 2
        g = c % 2
        heads = list(range(g * HPC, (g + 1) * HPC))
        sl = slice(g * OQ, (g + 1) * OQ)
        k_rows = np.concatenate(
            [W_kv[h * 2 * DH:h * 2 * DH + DH] for h in heads], 0)
        v_rows = np.concatenate(
            [W_kv[h * 2 * DH + DH:(h + 1) * 2 * DH] for h in heads], 0)
        bv_rows = np.concatenate(
            [b_kv[h * 2 * DH + DH:(h + 1) * 2 * DH] for h in heads], 0)
        xT = np.ascontiguousarray(x[b].T)      # [1024, 1024]
        encT = np.ascontiguousarray(enc[b].T)  # [1024, 2048]
        m = {
            "smf": np.concatenate(
                [np.stack([b_kv[h * 2 * DH:h * 2 * DH + DH] for h in heads],
                          1),
                 (b_q[sl] * scale).reshape(HPC, DH).T,
                 np.tile(bv_rows[None, :], (P, 1)),
                 np.eye(P)], axis=1).astype(np.float32),
            "ones": np.ones((P, 1), bf16),
        }
        for name, base, s in (("wk", _repack(k_rows.T), SW),
                              ("wv", _repack(v_rows.T), SW),
                              ("wq", _repack((W_q[sl] * scale).T), SQ),
                              ("wo", _repack(W_o[:, sl].T), SO)):
            hi, lo = _split8(base, s)
            m[name + "0"], m[name + "1"] = hi, lo
        for q in range(4):
            hi, lo = _split8(_repack(encT[:, q * NQ:(q + 1) * NQ]), SA)
            m[f"enc{q}0"], m[f"enc{q}1"] = hi, lo
        for q in range(2):
            hi, lo = _split8(_repack(xT[:, q * NQ:(q + 1) * NQ]), SA)
            m[f"x{q}0"], m[f"x{q}1"] = hi, lo
        if masked:
            m["maskT"] = np.ascontiguousarray(mask[b].T)
        in_maps.append(m)
    return in_maps


def kernel(mhca_input, encoder_output, cross_mask, W_kv, b_kv, W_q, b_q, W_o,
           b_o):
    from concourse.bass_utils import run_bass_kernel_spmd

    inputs = {
        "mhca_input": mhca_input, "encoder_output": encoder_output,
        "cross_mask": cross_mask, "W_kv": W_kv, "b_kv": b_kv, "W_q": W_q,
        "b_q": b_q, "W_o": W_o,
    }
    b_o = np.asarray(b_o, np.float32)
    masked = bool(np.any(np.asarray(cross_mask)))
    nc = _get_built(masked)
    in_maps = _shard_inputs(inputs, masked)

    res = run_bass_kernel_spmd(nc, in_maps, core_ids=list(range(N_CORES)))
    outs = [np.asarray(res.results[c]["out"], np.float32)
            for c in range(N_CORES)]
    full = np.stack([outs[2 * b] + outs[2 * b + 1] for b in range(B)], 0)
    return (full + b_o[None, None, :]).astype(np.float32)


# revision 27
# speedup vs baseline: 1.0039x; 1.0039x over previous
"""Multi-head cross-attention on 8 Trainium2 NeuronCores.

Problem shapes (hardcoded): B=4, Ld=1024, Le=2048, d_model=1024, 8 heads x 128.
Sharding: core c handles batch b=c//2 and head-group g=c%2 (4 heads each).
Each core computes q/k/v projections for its heads, attention, and a partial
output projection over its heads' value dims; the host sums the two partial
outputs per batch and adds b_o.

Projections run as split-fp8 DoubleRow matmuls: every operand is host-split
into hi+lo fp8e4 parts (hi = fp8(s*x), lo = fp8(s*x - hi), one power-of-2
scale s per tensor chosen for e4m3's normal range).  A DoubleRow matmul
contracts two 128-partition chunks per pass at 0.5 cycles/row, and the three
products hi*hi, hi*lo, lo*hi (lo*lo is ~2^-8 relative, dropped) cover a
d-chunk pair in 3 matmuls = 0.75x the bf16 cost at bf16-class accuracy.
PSUM drains rescale by the product of the operand scales (fused into the
bias-add on the DVE).  The output projection does the same with vals split
on-chip by the normalize chain.

All inputs are host-repacked into contiguous [128, ...] layouts so each
tensor is one DMA (descriptor-generation slots are an exclusive ~625ns
resource).  DMA order feeds the V path first (wk, enc, wv) with hi parts
before lo parts, giving the PE continuous early work while wq/x stream in.

Schedule: one software-pipelined stream; projection bank-groups (K/Q/V and
later the output projection) are queued a few per attention chunk-pair so
the PE never waits for the Act engine's exp stream.

Softmax denominators use tall-skinny matmuls: pT is the *stationary* operand
and a ones column moves, so each [128,1] per-query partial sum costs ~1 PE
row instead of the 512 a [1,512] ones-stationary layout costs.

Exps are issued per chunk-pair over a [128,1024] two-bank PSUM span (halving
the Act engine's fixed access overhead), double-buffered across two spans so
scores never wait on the previous exp.

The per-stage normalize chain (reciprocal -> PE-transpose to a [1,512] row
in a rotating projection bank -> gpsimd partition-broadcast -> scaled
multiply + fp8 hi/lo split of vals) is split: the PV accumulator is drained
to SBUF raw (one DVE copy) so the next stage's PV can start immediately, and
the rest is emitted after the next stage's first pair, off the PE's
critical path.

PSUM banks: 0-3 two score pair-spans, 4 PV accumulator, 5 denominator
columns, 6-7 projection / output-projection accumulators (rotating).
"""

import math
import sys

import numpy as np

for _p in ("/opt/trn_rl_repo", "/root/.axon_site/_ro/trn_rl_repo"):
    if _p not in sys.path:
        sys.path.append(_p)

B = 4
LQ = 1024
LK = 2048
D = 1024
H = 8
DH = 128
P = 128
HPC = 4          # heads per core
OQ = HPC * DH    # 512 projected dims per core
NQ = 512         # matmul moving free dim
KC = D // P      # 8 contraction chunks for projections
LKC = LK // P    # 16 key chunks
N_CORES = 8

# fp8e4 (e4m3) per-tensor scales: picked so values sit in the normal range.
SA = 8.0       # x, enc ~ N(0,1)
SW = 128.0     # wk, wv (sigma 1/32)
SQ = 1024.0    # wq with 1/sqrt(dh) folded (sigma 1/256)
SO = 128.0     # wo (sigma 1/32)
SVAL = 32.0    # vals (sigma ~0.03), applied on-chip
KINV = 1.0 / (SA * SW)
VINV = 1.0 / (SA * SW)
QINV = 1.0 / (SA * SQ)
OINV = 1.0 / (SVAL * SO)

_BUILT = {}


def _repack(a):
    """[KC*128, X] -> [128, KC*X] with d-chunk-major columns."""
    kc = a.shape[0] // P
    return np.ascontiguousarray(
        a.reshape(kc, P, a.shape[1]).transpose(1, 0, 2).reshape(P, -1))


def _split8(a, s):
    import ml_dtypes
    f8 = ml_dtypes.float8_e4m3
    sa = (a * s).astype(np.float32)
    hi = sa.astype(f8)
    lo = (sa - hi.astype(np.float32)).astype(f8)
    return hi, lo


def _build(masked):
    import concourse.bass as bass  # noqa: F401
    import concourse.tile as tile
    import concourse.mybir as mybir
    from concourse import bacc

    f32 = mybir.dt.float32
    bf16 = mybir.dt.bfloat16
    f32r = mybir.dt.float32r
    fp8 = mybir.dt.float8e4
    DR = mybir.MatmulPerfMode.DoubleRow
    Exp = mybir.ActivationFunctionType.Exp
    MUL = mybir.AluOpType.mult
    ADD = mybir.AluOpType.add
    SUB = mybir.AluOpType.subtract

    nc = bacc.Bacc("TRN2", target_bir_lowering=False, debug=False,
                   num_devices=N_CORES)

    def din(name, shape, dt=fp8):
        return nc.dram_tensor(name, shape, dt, kind="ExternalInput").ap()

    x_d = [[din(f"x{q}{p}", [P, KC, NQ]) for p in range(2)] for q in range(2)]
    e_d = [[din(f"enc{q}{p}", [P, KC, NQ]) for p in range(2)]
           for q in range(4)]
    wk_d = [din(f"wk{p}", [P, KC, OQ]) for p in range(2)]
    wv_d = [din(f"wv{p}", [P, KC, OQ]) for p in range(2)]
    wq_d = [din(f"wq{p}", [P, KC, OQ]) for p in range(2)]
    wo_d = [din(f"wo{p}", [P, HPC, D]) for p in range(2)]
    # bk | bq | bvb | ident packed as one f32 DMA.
    smf_d = din("smf", [P, 648], f32)
    ones_d = din("ones", [P, 1], bf16)
    if masked:
        maskT = din("maskT", [LK, LQ], f32)
    out_d = nc.dram_tensor("out", [LQ, D], bf16, kind="ExternalOutput").ap()

    with tile.TileContext(nc) as tc:
        with tc.tile_pool(name="persist", bufs=1) as persist:
            qT = [[persist.tile([P, NQ], f32r, name=f"qT{h}_{q}")
                   for q in range(2)] for h in range(HPC)]
            kT = [[persist.tile([P, NQ], f32r, name=f"kT{h}_{lk}")
                   for lk in range(4)] for h in range(HPC)]
            vch = [persist.tile([P, OQ], bf16, name=f"v{j}") for j in range(LKC)]
            smf = persist.tile([P, 648], f32, name="smf")
            bk_sb = smf[:, 0:HPC]
            bq_sb = smf[:, HPC:2 * HPC]
            bv_sb = smf[:, 2 * HPC:2 * HPC + OQ]
            ident = smf[:, 2 * HPC + OQ:2 * HPC + OQ + P]
            ones_col = persist.tile([P, 1], bf16, name="ones")
            warm = persist.tile([1, HPC], f32, name="warm")
            wkb = [persist.tile([P, KC, OQ], fp8, name=f"wkb{p}")
                   for p in range(2)]
            wvb = [persist.tile([P, KC, OQ], fp8, name=f"wvb{p}")
                   for p in range(2)]
            wqb = [persist.tile([P, KC, OQ], fp8, name=f"wqb{p}")
                   for p in range(2)]
            wob = [persist.tile([P, HPC, D], fp8, name=f"wob{p}")
                   for p in range(2)]
            eb = [[persist.tile([P, KC, NQ], fp8, name=f"eb{q}_{p}")
                   for p in range(2)] for q in range(4)]
            xb = [[persist.tile([P, KC, NQ], fp8, name=f"xb{q}_{p}")
                   for p in range(2)] for q in range(2)]
            vals = [[persist.tile([P, HPC, NQ], fp8, name=f"vals{q}_{p}")
                     for p in range(2)] for q in range(2)]

            with (
                tc.tile_pool(name="acc", bufs=1, space="PSUM") as acc,
                tc.tile_pool(name="pTp", bufs=3) as pTp,
                tc.tile_pool(name="smallp", bufs=2) as smallp,
                tc.tile_pool(name="maskp", bufs=16 if masked else 1) as maskp,
                tc.tile_pool(name="osb", bufs=4) as osb,
            ):
                # PSUM: two score pair-spans (banks 0-3), single PV
                # accumulator (bank 4), denominator columns (bank 5),
                # projection/out-proj accumulators (banks 6-7).
                sp = [acc.tile([P, 2 * NQ], f32, name=f"sp{t}")
                      for t in range(2)]
                pvb = acc.tile([P, NQ], f32, name="pvb")
                dbk = acc.tile([P, NQ], f32, name="dbk")
                pj = [acc.tile([P, NQ], f32, name=f"pj{t}") for t in range(2)]

                # Warm-up fill: [1,512]-out matmuls against a memset tile (no
                # DMA dependency) keep the PE busy through the DMA lead-in
                # and hold the p-state ramp continuous; the Act Exp table is
                # preloaded the same way.
                wt = persist.tile([P, NQ], bf16, name="wt")
                nc.vector.memset(wt[:], 1.0)

                def warmfill(n):
                    for _ in range(n):
                        nc.tensor.matmul(pj[1][:1, :], wt[:, :1], wt[:],
                                         start=True, stop=True)

                warmfill(2)
                nc.scalar.activation(warm[:], wt[:1, :HPC], Exp)
                # ---- DMA issue order == service order: V path first,
                # hi parts before lo parts.
                nc.sync.dma_start(wkb[0][:], wk_d[0][:])
                nc.sync.dma_start(eb[0][0][:], e_d[0][0][:])
                nc.sync.dma_start(wkb[1][:], wk_d[1][:])
                nc.sync.dma_start(eb[0][1][:], e_d[0][1][:])
                for p in range(2):
                    nc.sync.dma_start(wvb[p][:], wv_d[p][:])
                nc.sync.dma_start(smf[:], smf_d[:])
                nc.sync.dma_start(ones_col[:], ones_d[:])
                for p in range(2):
                    nc.sync.dma_start(eb[1][p][:], e_d[1][p][:])
                for p in range(2):
                    nc.sync.dma_start(wqb[p][:], wq_d[p][:])
                for p in range(2):
                    nc.sync.dma_start(xb[0][p][:], x_d[0][p][:])
                for p in range(2):
                    nc.sync.dma_start(eb[2][p][:], e_d[2][p][:])
                for p in range(2):
                    nc.sync.dma_start(eb[3][p][:], e_d[3][p][:])
                for p in range(2):
                    nc.sync.dma_start(xb[1][p][:], x_d[1][p][:])
                for p in range(2):
                    nc.sync.dma_start(wob[p][:], wo_d[p][:])

                # ---- projection bank-group emitters (banks 6-7 rotating).
                # Each contraction d-chunk pair takes 3 DoubleRow matmuls:
                # hi*hi, lo(w)*hi, hi*lo (lo*lo dropped).  The hi*hi pass for
                # all pairs is emitted first so it can start as soon as the
                # hi DMAs land.
                nbg = [0]

                def next_pj():
                    bank = pj[nbg[0] % 2]
                    nbg[0] += 1
                    return bank

                def dr_group(bank, wts, mov, wslice, mslice, between=None):
                    """wts/mov: [hi, lo] tile lists; slices by d-pair dp."""
                    np_ = KC // 2
                    for wp, mp, first, last in ((0, 0, True, False),
                                                (1, 0, False, False),
                                                (0, 1, False, True)):
                        for dp in range(np_):
                            nc.tensor.matmul(
                                bank[:],
                                wts[wp][wslice(dp)],
                                mov[mp][mslice(dp)],
                                start=(first and dp == 0),
                                stop=(last and dp == np_ - 1),
                                perf_mode=DR)
                        if between is not None and not last:
                            between()

                def kproj_group(h, lk, between=None):
                    bank = next_pj()
                    dr_group(
                        bank, wkb, eb[lk],
                        lambda dp: np.s_[:, 2 * dp:2 * dp + 2,
                                         h * DH:(h + 1) * DH],
                        lambda dp: np.s_[:, 2 * dp:2 * dp + 2, :],
                        between=between)
                    nc.vector.tensor_scalar(
                        kT[h][lk][:], bank[:], KINV, bk_sb[:, h:h + 1],
                        MUL, ADD)

                def qproj_group(h, q2):
                    bank = next_pj()
                    dr_group(
                        bank, wqb, xb[q2],
                        lambda dp: np.s_[:, 2 * dp:2 * dp + 2,
                                         h * DH:(h + 1) * DH],
                        lambda dp: np.s_[:, 2 * dp:2 * dp + 2, :])
                    nc.vector.tensor_scalar(
                        qT[h][q2][:], bank[:], QINV, bq_sb[:, h:h + 1],
                        MUL, ADD)

                def vproj_group(j):
                    bank = next_pj()
                    dr_group(
                        bank, eb[j // 4], wvb,
                        lambda dp: np.s_[:, 2 * dp:2 * dp + 2,
                                         (j % 4) * P:(j % 4 + 1) * P],
                        lambda dp: np.s_[:, 2 * dp:2 * dp + 2, :])
                    nc.vector.scalar_tensor_tensor(
                        vch[j][:], bank[:], VINV, bv_sb[:], MUL, ADD)

                def oproj_group(lqc, o2, n):
                    bank = next_pj()
                    q2g = lqc // 4
                    for wp, mp, first, last in ((0, 0, True, False),
                                                (1, 0, False, False),
                                                (0, 1, False, True)):
                        for hp in range(HPC // 2):
                            nc.tensor.matmul(
                                bank[:],
                                vals[q2g][wp][:, 2 * hp:2 * hp + 2,
                                              (lqc % 4) * P:
                                              (lqc % 4 + 1) * P],
                                wob[mp][:, 2 * hp:2 * hp + 2,
                                        o2 * NQ:(o2 + 1) * NQ],
                                start=(first and hp == 0),
                                stop=(last and hp == HPC // 2 - 1),
                                perf_mode=DR)
                    ot = osb.tile([P, NQ], bf16, name="ot")
                    if n % 2 == 0:
                        nc.vector.tensor_scalar_mul(ot[:], bank[:], OINV)
                    else:
                        nc.scalar.mul(ot[:], bank[:], OINV)
                    nc.sync.dma_start(
                        out_d[lqc * P:(lqc + 1) * P,
                              o2 * NQ:(o2 + 1) * NQ], ot[:])

                # ---- attention emitters
                mask_tiles = [None] * LKC

                def attn_pair(q2, h, jp):
                    """Chunks j=2jp,2jp+1: scores into pair-span jp%2, one
                    exp over both, then PV + denominator matmuls."""
                    span = sp[jp % 2]
                    for t in range(2):
                        j = 2 * jp + t
                        nc.tensor.matmul(
                            span[:, t * NQ:(t + 1) * NQ],
                            kT[h][j // 4][:, (j % 4) * P:(j % 4 + 1) * P],
                            qT[h][q2][:],
                            start=True, stop=True)
                        if masked:
                            nc.vector.tensor_add(
                                span[:, t * NQ:(t + 1) * NQ],
                                span[:, t * NQ:(t + 1) * NQ],
                                mask_tiles[j][:])
                    pT2 = pTp.tile([P, 2 * NQ], bf16, name="pT2")
                    nc.scalar.activation(pT2[:], span[:], Exp)
                    for t in range(2):
                        j = 2 * jp + t
                        nc.tensor.matmul(
                            pvb[:],
                            vch[j][:, h * DH:(h + 1) * DH],
                            pT2[:, t * NQ:(t + 1) * NQ],
                            start=(j == 0), stop=(j == LKC - 1))
                        for s in range(4):
                            nc.tensor.matmul(
                                dbk[:, s:s + 1],
                                pT2[:, t * NQ + s * P:t * NQ + (s + 1) * P],
                                ones_col[:],
                                start=(j == 0 and s == 0),
                                stop=(j == LKC - 1 and s == 3),
                                skip_group_check=True)

                def attn_norm_start(q2, h):
                    """Free the PV/denominator banks: raw-copy the PV
                    accumulator and take the reciprocal of d."""
                    pvraw = smallp.tile([P, NQ], f32, name="pvraw")
                    nc.vector.tensor_copy(pvraw[:], pvb[:])
                    rsb = smallp.tile([P, 4], f32, name="rsb")
                    nc.vector.reciprocal(rsb[:], dbk[:, 0:4])
                    return pvraw, rsb

                def attn_norm_finish(q2, h, pvraw, rsb):
                    """1/d -> [1,512] row via PE transposes (into a rotating
                    projection bank) -> partition broadcast -> scaled
                    normalize, split into hi+lo fp8 for the out-proj."""
                    tb = next_pj()
                    for s in range(4):
                        nc.tensor.transpose(
                            tb[0:1, s * P:(s + 1) * P],
                            rsb[:, s:s + 1], ident[:])
                    rrow = smallp.tile([1, NQ], f32, name="rrow")
                    nc.vector.tensor_copy(rrow[:], tb[0:1, :])
                    bcast = smallp.tile([P, NQ], f32, name="bcast")
                    nc.gpsimd.partition_broadcast(bcast[:], rrow[:])
                    t2 = smallp.tile([P, NQ], f32, name="t2")
                    nc.vector.scalar_tensor_tensor(
                        t2[:], pvraw[:], SVAL, bcast[:], MUL, MUL)
                    vh = vals[q2][0][:, h:h + 1, :]
                    nc.vector.tensor_copy(vh, t2[:])
                    nc.vector.scalar_tensor_tensor(
                        vals[q2][1][:, h:h + 1, :], t2[:], 1.0, vh, MUL, SUB)

                # ---- the pipelined schedule.
                # Pre-stage: first K group, the V stream, then Q(h0,0), with
                # warm-up fill sized to the DMA lead-in.
                warmfill(10)
                kproj_group(0, 0, between=lambda: warmfill(1))
                warmfill(2)
                for j in range(8):
                    vproj_group(j)
                qproj_group(0, 0)

                K, Q, V = kproj_group, qproj_group, vproj_group

                def O(n):
                    return lambda: oproj_group(n // 2, n % 2, n)

                def L(f, *a):
                    return lambda: f(*a)

                # Per-stage, per-pair-slot work placement.  Projections lead
                # the attention that consumes them; out-proj groups go at
                # jp>=5 of their earliest stage or jp=0 of later stages so
                # the producing normalize chain has always landed; each
                # attention-only stage start gets one filler group to cover
                # the exp pipeline-fill bubble.
                stage_work = {
                    0: {0: [L(K, 0, 1), L(V, 8)], 1: [L(V, 9), L(K, 0, 2)],
                        2: [L(V, 10), L(V, 11)], 3: [L(K, 0, 3), L(V, 12)],
                        4: [L(V, 13), L(K, 1, 0)], 5: [L(V, 14), L(V, 15)],
                        6: [L(K, 1, 1), L(K, 1, 2)],
                        7: [L(K, 1, 3), L(Q, 1, 0)]},
                    1: {0: [L(Q, 0, 1)], 1: [L(Q, 1, 1)], 2: [L(K, 2, 0)],
                        3: [L(K, 2, 1)], 4: [L(K, 2, 2)], 5: [L(K, 2, 3)],
                        6: [L(Q, 2, 0)]},
                    2: {0: [L(K, 3, 0)], 1: [L(K, 3, 1)], 2: [L(K, 3, 2)],
                        3: [L(K, 3, 3)], 4: [L(Q, 3, 0)]},
                    3: {0: [L(Q, 2, 1)]},
                    4: {0: [L(Q, 3, 1)], 5: [O(0)], 6: [O(1)]},
                    5: {0: [O(2)], 5: [O(3)]},
                    6: {0: [O(4)], 5: [O(5)]},
                    7: {0: [O(6)], 5: [O(7)]},
                }

                pending = None
                for idx in range(8):
                    q2, h = idx // 4, idx % 4
                    if masked and h == 0:
                        for j in range(LKC):
                            mt = maskp.tile([P, NQ], f32, name=f"m{j}")
                            nc.sync.dma_start(
                                mt[:],
                                maskT[j * P:(j + 1) * P,
                                      q2 * NQ:(q2 + 1) * NQ])
                            mask_tiles[j] = mt
                    work = stage_work.get(idx, {})
                    for jp in range(LKC // 2):
                        for w in work.get(jp, []):
                            w()
                        attn_pair(q2, h, jp)
                        if jp == 0 and pending is not None:
                            # finish the previous stage's normalize off the
                            # critical path
                            attn_norm_finish(*pending)
                            pending = None
                    pvraw, rsb = attn_norm_start(q2, h)
                    pending = (q2, h, pvraw, rsb)

                attn_norm_finish(*pending)
                # Tail: second-half output projection.
                for n in range(8, 16):
                    oproj_group(n // 2, n % 2, n)

    nc.compile()
    return nc


def _get_built(masked):
    if masked not in _BUILT:
        _BUILT[masked] = _build(masked)
    return _BUILT[masked]


def _shard_inputs(inputs, masked):
    import ml_dtypes

    bf16 = ml_dtypes.bfloat16

    x = np.asarray(inputs["mhca_input"], np.float32)
    enc = np.asarray(inputs["encoder_output"], np.float32)
    mask = np.asarray(inputs["cross_mask"], np.float32)
    W_kv = np.asarray(inputs["W_kv"], np.float32)
    b_kv = np.asarray(inputs["b_kv"], np.float32)
    W_q = np.asarray(inputs["W_q"], np.float32)
    b_q = np.asarray(inputs["b_q"], np.float32)
    W_o = np.asarray(inputs["W_o"], np.float32)

    scale = 1.0 / math.sqrt(DH)
    in_maps = []
    for c in range(N_CORES):
        b = c // 2
        g = c % 2
        heads = list(range(g * HPC, (g + 1) * HPC))
        sl = slice(g * OQ, (g + 1) * OQ)
        k_rows = np.concatenate(
            [W_kv[h * 2 * DH:h * 2 * DH + DH] for h in heads], 0)
        v_rows = np.concatenate(
            [W_kv[h * 2 * DH + DH:(h + 1) * 2 * DH] for h in heads], 0)
        bv_rows = np.concatenate(
            [b_kv[h * 2 * DH + DH:(h + 1) * 2 * DH] for h in heads], 0)
        xT = np.ascontiguousarray(x[b].T)      # [1024, 1024]
        encT = np.ascontiguousarray(enc[b].T)  # [1024, 2048]
        m = {
            "smf": np.concatenate(
                [np.stack([b_kv[h * 2 * DH:h * 2 * DH + DH] for h in heads],
                          1),
                 (b_q[sl] * scale).reshape(HPC, DH).T,
                 np.tile(bv_rows[None, :], (P, 1)),
                 np.eye(P)], axis=1).astype(np.float32),
            "ones": np.ones((P, 1), bf16),
        }
        for name, base, s in (("wk", _repack(k_rows.T), SW),
                              ("wv", _repack(v_rows.T), SW),
                              ("wq", _repack((W_q[sl] * scale).T), SQ),
                              ("wo", _repack(W_o[:, sl].T), SO)):
            hi, lo = _split8(base, s)
            m[name + "0"], m[name + "1"] = hi, lo
        for q in range(4):
            hi, lo = _split8(_repack(encT[:, q * NQ:(q + 1) * NQ]), SA)
            m[f"enc{q}0"], m[f"enc{q}1"] = hi, lo
        for q in range(2):
            hi, lo = _split8(_repack(xT[:, q * NQ:(q + 1) * NQ]), SA)
            m[f"x{q}0"], m[f"x{q}1"] = hi, lo
        if masked:
            m["maskT"] = np.ascontiguousarray(mask[b].T)
        in_maps.append(m)
    return in_maps


def kernel(mhca_input, encoder_output, cross_mask, W_kv, b_kv, W_q, b_q, W_o,
           b_o):
    from concourse.bass_utils import run_bass_kernel_spmd

    inputs = {
        "mhca_input": mhca_input, "encoder_output": encoder_output,
        "cross_mask": cross_mask, "W_kv": W_kv, "b_kv": b_kv, "W_q": W_q,
        "b_q": b_q, "W_o": W_o,
    }
    b_o = np.asarray(b_o, np.float32)
    masked = bool(np.any(np.asarray(cross_mask)))
    nc = _get_built(masked)
    in_maps = _shard_inputs(inputs, masked)

    res = run_bass_kernel_spmd(nc, in_maps, core_ids=list(range(N_CORES)))
    outs = [np.asarray(res.results[c]["out"], np.float32)
            for c in range(N_CORES)]
    full = np.stack([outs[2 * b] + outs[2 * b + 1] for b in range(B)], 0)
    return (full + b_o[None, None, :]).astype(np.float32)
